# revision 1
# baseline (speedup 1.0000x reference)
"""Trainium2 Bass kernel for nn_DynaResidualBlock (hypernetwork residual block).

Reference computation (B=32, LAT=256, FIN=FOUT=32, FH=64, H=W=128):
    h  = lat @ W1 + b1                       # [B, 9408]
    ks = h @ W2 + b2                         # [B, 9408]  (W2 is 9408x9408)
    per-sample 1x1 convs with kernels/biases sliced out of ks:
    x_s = k_short(x) ; y = k_out(lrelu(k_mid(lrelu(k_in(x))))) + x_s

Sharding over 8 cores:
  - hypernet contraction dim (9408) split 1176-per-core: core i holds
    W1[:, shard_i] and W2[shard_i, :] (cast to fp16 host-side) and computes
    a partial ks for ALL 32 samples; per-segment ReduceScatter collectives
    hand core i the summed ks rows for its own 4 samples.
  - conv phase is data-parallel: core i processes samples 4i..4i+3 packed
    as x4 [128, HW] (4 samples x 32 channels); each conv stage runs as
    per-sample small matmuls packed into disjoint 32x32 PE sub-arrays via
    tile_position, so a stage costs ~one matmul of 512 columns.

Overlap structure: W2 columns are permuted host-side so the generated ks
arrives segment-by-segment in phase-B consumption order
(biases+k_in | k_mid | k_short+k_out). Conv stages 1-2 are emitted
interleaved with the W2 streaming loop so they fill PE gaps during the
(DMA-bound) hypernet phase; only stage 3/4 + output stores trail the
final ReduceScatter.
"""

import contextlib

import numpy as np

import concourse.bacc as bacc
import concourse.mybir as mybir
import concourse.tile as tile
from concourse.bass_utils import run_bass_kernel_spmd

N_CORES = 8
B, LAT, FIN, FOUT, FH, H, W = 32, 256, 32, 32, 64, 128, 128
HW = H * W
K_IN, K_MID, K_OUT, K_SH = FH * FIN, FH * FH, FOUT * FH, FOUT * FIN
K_TOT = K_IN + K_MID + K_OUT + K_SH + FH + FH + FOUT + FOUT  # 9408
SHARD = K_TOT // N_CORES  # 1176 hypernet contraction rows per core
KP = SHARD + 1            # + one bias row (b2, on core 0 only)
KPAD = 1280               # h length padded to 10 chunks of 128
NCH = KPAD // 128         # 10
BPC = B // N_CORES        # 4 samples per core
NW = HW // 512            # 32 conv pixel windows
F32 = mybir.dt.float32
FP16 = mybir.dt.float16

# original ks offsets
OFF_IN, OFF_MID = 0, K_IN
OFF_OUT, OFF_SHC = K_IN + K_MID, K_IN + K_MID + K_OUT
OFF_B = OFF_SHC + K_SH  # 9216: b_in 64 | b_mid 64 | b_out 32 | b_short 32

# permuted ks layout: | biases 192 | k_inT 2048 | k_midT 4096 | k_shortT 1024
# | k_outT 2048 | — segments ordered by phase-B consumption so each
# ReduceScatter fires as early as possible.
NB_IN, NB_MID, NB_OUT, NB_SH = 0, 64, 128, 160
NK_IN, NK_MID, NK_SH, NK_OUT = 192, 2240, 6336, 7360
SEGS = [(0, 2240), (2240, 4096), (6336, 3072)]

_CACHE: dict = {}


def _strips(wseg):
    return [(t, min(512, wseg - t)) for t in range(0, wseg, 512)]


def _build():
    nc = bacc.Bacc("TRN2", target_bir_lowering=False, num_devices=N_CORES)
    AF = mybir.ActivationFunctionType

    xs = nc.dram_tensor("xs", [BPC * FIN, HW], FP16, kind="ExternalInput")
    latT = nc.dram_tensor("latT", [LAT, B], FP16, kind="ExternalInput")
    w1s = nc.dram_tensor("w1s", [LAT, KPAD], FP16, kind="ExternalInput")
    b1s = nc.dram_tensor("b1s", [128, NCH], F32, kind="ExternalInput")
    w2s = nc.dram_tensor("w2s", [KP, K_TOT], FP16, kind="ExternalInput")
    out = nc.dram_tensor("out", [BPC * FOUT, HW], F32, kind="ExternalOutput")

    rs_in = [nc.dram_tensor(f"rs_in{s}", [B, w], FP16)
             for s, (_, w) in enumerate(SEGS)]
    ks_own = [nc.dram_tensor(f"ks_own{s}", [BPC, w], FP16)
              for s, (_, w) in enumerate(SEGS)]

    with tile.TileContext(nc) as tc, contextlib.ExitStack() as ctx:
        sing = ctx.enter_context(tc.tile_pool(name="sing", bufs=1))
        w2pool = ctx.enter_context(tc.tile_pool(name="w2p", bufs=3))
        kspool = ctx.enter_context(tc.tile_pool(name="ksp", bufs=2))
        outp = ctx.enter_context(tc.tile_pool(name="outp", bufs=4))
        # PSUM budget is 8 banks of [128, 512] f32; bufs is per-tag:
        # psK(pk)=2 + ps1(p1,p1b)=2 + ps2(p2,p2b)=2 + psA(pA)=2 = 8.
        psK = ctx.enter_context(tc.tile_pool(name="psK", bufs=2, space="PSUM"))
        ps1 = ctx.enter_context(tc.tile_pool(name="ps1", bufs=1, space="PSUM"))
        ps2 = ctx.enter_context(tc.tile_pool(name="ps2", bufs=1, space="PSUM"))
        psA = ctx.enter_context(tc.tile_pool(name="psA", bufs=2, space="PSUM"))

        latT_sb = sing.tile([128, 2 * B], FP16)
        w1_sb = sing.tile([128, 2 * KPAD], FP16)
        b1_sb = sing.tile([128, NCH], F32)
        hT_sb = sing.tile([128, NCH * B], FP16)
        x4 = sing.tile([128, HW], FP16)
        y1all = sing.tile([128, 2 * HW], FP16)
        y2all = sing.tile([128, 2 * HW], FP16)

        # phase-B weight/bias tiles (loaded from ks_own after each RS)
        kin_all = sing.tile([128, FH], FP16)
        ksh_all = sing.tile([128, FOUT], FP16)
        kmid = [sing.tile([128, FH], FP16, name=f"kmid{p}") for p in range(2)]
        kout = [sing.tile([128, FOUT], FP16, name=f"kout{p}") for p in range(2)]
        bin_v = [sing.tile([128, 1], F32, name=f"bin{p}") for p in range(2)]
        bmid_v = [sing.tile([128, 1], F32, name=f"bmid{p}") for p in range(2)]
        bout4 = sing.tile([128, 1], F32)
        bsh4 = sing.tile([128, 1], F32)
        bso = sing.tile([128, 1], F32)

        for l in range(2):
            nc.sync.dma_start(out=latT_sb[:, l * B:(l + 1) * B],
                              in_=latT[128 * l:128 * (l + 1), :])
            nc.sync.dma_start(out=w1_sb[:, l * KPAD:(l + 1) * KPAD],
                              in_=w1s[128 * l:128 * (l + 1), :])
        nc.sync.dma_start(out=b1_sb[:, :], in_=b1s[:, :])

        # ---- hypernet stage 1: hT[kcol, b] = sum_l W1[l, kcol]*lat[b, l] ----
        for c in range(NCH):
            ph = psA.tile([128, 512], F32, tag="pA", name=f"ph{c}")
            for l in range(2):
                nc.tensor.matmul(
                    ph[:, 0:B],
                    lhsT=w1_sb[:, l * KPAD + 128 * c: l * KPAD + 128 * c + 128],
                    rhs=latT_sb[:, l * B:(l + 1) * B],
                    start=(l == 0), stop=(l == 1))
            nc.scalar.activation(out=hT_sb[:, c * B:(c + 1) * B], in_=ph[:, 0:B],
                                 func=AF.Identity, bias=b1_sb[:, c:c + 1],
                                 scale=1.0)

        # ---- phase-B emission helpers (interleaved into the W2 stream) ----
        def emit_s1(w):
            c0 = 512 * w
            p1 = ps1.tile([128, 512], F32, tag="p1", name=f"p1_{w}")
            p1b = ps1.tile([128, 512], F32, tag="p1b", name=f"p1b_{w}")
            for q in range(4):
                pt = p1 if q < 2 else p1b
                hq = q % 2
                nc.tensor.matmul(
                    pt[64 * hq:64 * hq + 64, :],
                    lhsT=kin_all[32 * q:32 * q + 32, :],
                    rhs=x4[32 * q:32 * q + 32, c0:c0 + 512],
                    start=True, stop=True, tile_position=(32 * q, 64 * hq))
            yc0 = 1024 * w
            nc.scalar.activation(out=y1all[:, yc0:yc0 + 512], in_=p1,
                                 func=AF.Lrelu,
                                 bias=bin_v[0][:, 0:1], scale=1.0, alpha=0.01)
            nc.scalar.activation(out=y1all[:, yc0 + 512:yc0 + 1024], in_=p1b,
                                 func=AF.Lrelu,
                                 bias=bin_v[1][:, 0:1], scale=1.0, alpha=0.01)

        def emit_s2(w):
            p2 = ps2.tile([128, 512], F32, tag="p2", name=f"p2_{w}")
            p2b = ps2.tile([128, 512], F32, tag="p2b", name=f"p2b_{w}")
            for q in range(4):
                p = q // 2
                hq = q % 2
                pt = p2 if p == 0 else p2b
                nc.tensor.matmul(
                    pt[64 * hq:64 * hq + 64, :],
                    lhsT=kmid[p][64 * hq:64 * hq + 64, :],
                    rhs=y1all[64 * hq:64 * hq + 64,
                              1024 * w + 512 * p:1024 * w + 512 * p + 512],
                    start=True, stop=True, tile_position=(64 * hq, 64 * hq))
            yc0 = 1024 * w
            nc.scalar.activation(out=y2all[:, yc0:yc0 + 512], in_=p2,
                                 func=AF.Lrelu, bias=bmid_v[0][:, 0:1],
                                 scale=1.0, alpha=0.01)
            nc.scalar.activation(out=y2all[:, yc0 + 512:yc0 + 1024], in_=p2b,
                                 func=AF.Lrelu, bias=bmid_v[1][:, 0:1],
                                 scale=1.0, alpha=0.01)

        # ---- hypernet stage 2: stream W2 (fp16), partial ks, ReduceScatter --
        # All W2/x DMAs are emitted BEFORE any conv-phase work so neither HWDGE
        # queue ever stalls behind an RS-gated activation or matmul. RS
        # triggers are emitted ahead of the weight loads that wait on the
        # previous RS, so the in-order gpsimd queue never delays a trigger.
        for s, (o0, wseg) in enumerate(SEGS):
            if s == 1:
                for xc in range(8):
                    nc.sync.dma_start(out=x4[:, 2048 * xc:2048 * (xc + 1)],
                                      in_=xs[:, 2048 * xc:2048 * (xc + 1)])
            strips = _strips(wseg)
            nbank = (len(strips) + 3) // 4
            banks = [psK.tile([128, 512], F32, tag="pk", name=f"pk_{s}_{bi}")
                     for bi in range(nbank)]
            for c in range(NCH):
                kc = 128 if c < NCH - 1 else KP - 128 * (NCH - 1)
                w2t = w2pool.tile([128, wseg], FP16, tag="w2t",
                                  name=f"w2t_{s}_{c}")
                eng = nc.scalar if c % 3 == 2 else nc.sync
                eng.dma_start(out=w2t[0:kc, :],
                              in_=w2s[128 * c:128 * c + kc, o0:o0 + wseg])
                for j, (t0, tw) in enumerate(strips):
                    bank, sib = banks[j // 4], j % 4
                    nc.tensor.matmul(
                        bank[32 * sib:32 * sib + 32, 0:tw],
                        lhsT=hT_sb[0:kc, c * B:(c + 1) * B],
                        rhs=w2t[0:kc, t0:t0 + tw],
                        start=(c == 0), stop=(c == NCH - 1),
                        tile_position=(0, 32 * sib))
            for bi, bank in enumerate(banks):
                bw = min(512, wseg - 2048 * bi)
                ks16 = kspool.tile([128, 512], FP16, tag="ks16",
                                   name=f"ks16_{s}_{bi}")
                nc.vector.tensor_copy(ks16[:, 0:bw], bank[:, 0:bw])
                for j, (t0, tw) in enumerate(strips):
                    if j // 4 != bi:
                        continue
                    nc.sync.dma_start(
                        out=rs_in[s][:, t0:t0 + tw],
                        in_=ks16[32 * (j % 4):32 * (j % 4) + 32, 0:tw])
            nc.gpsimd.collective_compute(
                "ReduceScatter", mybir.AluOpType.add,
                replica_groups=[list(range(N_CORES))],
                ins=[rs_in[s][:, :].opt()], outs=[ks_own[s][:, :].opt()])

        # phase-B weight loads, after ALL RS triggers on the gpsimd queue
        nc.gpsimd.dma_start(
            out=kin_all,
            in_=ks_own[0][:, NK_IN:NK_IN + K_IN]
            .rearrange("q (i o) -> q i o", i=FIN))
        for p in range(2):
            nc.gpsimd.dma_start(
                out=bin_v[p][:, 0:1],
                in_=ks_own[0][2 * p:2 * p + 2, NB_IN:NB_IN + FH])
        for p in range(2):
            nc.gpsimd.dma_start(
                out=kmid[p],
                in_=ks_own[1][2 * p:2 * p + 2, :]
                .rearrange("q (i o) -> q i o", i=FH))
            nc.gpsimd.dma_start(
                out=bmid_v[p][:, 0:1],
                in_=ks_own[0][2 * p:2 * p + 2, NB_MID:NB_MID + FH])

        # ---- conv stages 1-2 for all windows (overlap RS1/RS2 latency) ----
        for w in range(NW):
            emit_s1(w)
        for w in range(NW):
            emit_s2(w)

        # seg-2 weight loads ride the (idle) sync queue at the tail
        nc.sync.dma_start(out=ksh_all,
                          in_=ks_own[2][:, 0:K_SH]
                          .rearrange("q (i o) -> q i o", i=FIN))
        for p in range(2):
            nc.sync.dma_start(out=kout[p],
                              in_=ks_own[2][2 * p:2 * p + 2, K_SH:K_SH + K_OUT]
                              .rearrange("q (i o) -> q i o", i=FH))
        nc.gpsimd.dma_start(out=bout4[:, 0:1],
                            in_=ks_own[0][:, NB_OUT:NB_OUT + FOUT])
        nc.gpsimd.dma_start(out=bsh4[:, 0:1],
                            in_=ks_own[0][:, NB_SH:NB_SH + FOUT])
        nc.vector.tensor_add(bso, bout4, bsh4)

        # keep the PE warm through the RS2 wait so stage-3/4 runs at 2.4 GHz
        warm = psK.tile([128, 512], F32, tag="pk", name="warm")
        for d in range(24):
            nc.tensor.matmul(warm[0:64, :], lhsT=kmid[0][0:64, :],
                             rhs=y2all[0:64, 0:512], start=True, stop=True,
                             tile_position=(0, 0))

        # ---- conv stage 3 (k_out) + stage 4 (k_short) + output ----
        for w in range(NW):
            c0 = 512 * w
            yc0 = 1024 * w
            p3 = psA.tile([128, 512], F32, tag="pA", name=f"p3_{w}")
            for q in range(4):
                p = q // 2
                hq = q % 2
                nc.tensor.matmul(
                    p3[32 * q:32 * q + 32, :],
                    lhsT=kout[p][64 * hq:64 * hq + 64, :],
                    rhs=y2all[64 * hq:64 * hq + 64,
                              yc0 + 512 * p:yc0 + 512 * p + 512],
                    start=True, stop=False, tile_position=(64 * hq, 32 * q))
            for q in range(4):
                nc.tensor.matmul(
                    p3[32 * q:32 * q + 32, :],
                    lhsT=ksh_all[32 * q:32 * q + 32, :],
                    rhs=x4[32 * q:32 * q + 32, c0:c0 + 512],
                    start=False, stop=True, tile_position=(32 * q, 32 * q))
            o_sb = outp.tile([128, 512], F32, tag="o", name=f"o_{w}")
            nc.scalar.activation(out=o_sb, in_=p3, func=AF.Identity,
                                 bias=bso[:, 0:1], scale=1.0)
            eng = nc.sync if w % 2 == 0 else nc.gpsimd
            eng.dma_start(out=out[:, c0:c0 + 512], in_=o_sb)

    nc.compile()
    return nc


def _seg_perm(rows, cols):
    # new position (c, r) holds old flat index r*cols + c
    return np.arange(rows * cols).reshape(rows, cols).T.ravel()


def _perm():
    # permutation of ks columns: conv kernels arrive transposed (lhsT layout)
    # and segments are reordered to the phase-B consumption order
    return np.concatenate([
        np.arange(OFF_B, K_TOT),          # biases first
        OFF_IN + _seg_perm(FH, FIN),      # k_inT
        OFF_MID + _seg_perm(FH, FH),      # k_midT
        OFF_SHC + _seg_perm(FOUT, FIN),   # k_shortT
        OFF_OUT + _seg_perm(FOUT, FH),    # k_outT
    ])


def _prep_in_maps(x, lat, W1, b1, W2, b2):
    x = np.ascontiguousarray(x, np.float32)
    lat = np.ascontiguousarray(lat, np.float32)
    W1 = np.ascontiguousarray(W1, np.float32)
    b1 = np.asarray(b1, np.float32)
    W2 = np.asarray(W2, np.float32)
    b2 = np.asarray(b2, np.float32)

    perm = _perm()
    W2p = np.ascontiguousarray(W2[:, perm]).astype(np.float16)
    b2p = b2[perm].astype(np.float16)
    latT = np.ascontiguousarray(lat.T).astype(np.float16)
    xr = x.reshape(B, FIN, HW)

    in_maps = []
    for i in range(N_CORES):
        sh = slice(i * SHARD, (i + 1) * SHARD)
        w1p = np.zeros((LAT, KPAD), np.float16)
        w1p[:, :SHARD] = W1[:, sh]
        b1p = np.zeros((KPAD,), np.float32)
        b1p[:SHARD] = b1[sh]
        b1p[SHARD] = 1.0  # the "ones" h-slot that carries b2
        w2a = np.zeros((KP, K_TOT), np.float16)
        w2a[:SHARD] = W2p[sh]
        if i == 0:
            w2a[SHARD] = b2p
        xsi = np.ascontiguousarray(
            xr[4 * i:4 * i + 4].reshape(BPC * FIN, HW)).astype(np.float16)
        in_maps.append({
            "xs": xsi,
            "latT": latT,
            "w1s": w1p,
            "b1s": np.ascontiguousarray(b1p.reshape(NCH, 128).T),
            "w2s": w2a,
        })
    return in_maps


def _run(in_maps, **kwargs):
    if "nc" not in _CACHE:
        _CACHE["nc"] = _build()
    return run_bass_kernel_spmd(_CACHE["nc"], in_maps,
                                core_ids=list(range(N_CORES)), **kwargs)


def _assemble(results):
    parts = [r["out"].reshape(BPC, FOUT, H, W) for r in results]
    return np.ascontiguousarray(np.concatenate(parts, axis=0))


def kernel(x, lat, W1, b1, W2, b2):
    in_maps = _prep_in_maps(x, lat, W1, b1, W2, b2)
    res = _run(in_maps)
    return _assemble(res.results)



# revision 4
# speedup vs baseline: 1.3263x; 1.3263x over previous
"""Trainium2 Bass kernel for nn_DynaResidualBlock (hypernetwork residual block).

Reference computation (B=32, LAT=256, FIN=FOUT=32, FH=64, H=W=128):
    h  = lat @ W1 + b1                       # [B, 9408]
    ks = h @ W2 + b2                         # [B, 9408]  (W2 is 9408x9408)
    per-sample 1x1 convs with kernels/biases sliced out of ks:
    x_s = k_short(x) ; y = k_out(lrelu(k_mid(lrelu(k_in(x))))) + x_s

Sharding over 8 cores:
  - hypernet contraction dim (9408) split 1176-per-core: core i holds
    W1[:, shard_i] and W2[shard_i, :] (fp16, host-cast) and computes a
    partial ks for all 32 samples; per-segment ReduceScatter collectives
    hand core i the summed ks rows for its own 4 samples.
  - conv phase is data-parallel: core i processes samples 4i..4i+3 packed
    as x4 [128, HW].  Each conv stage is a block-diagonal full-array
    matmul: per 512-px window, stage1 = 2 instrs ([128,128] lhsT with two
    32x64 sample blocks), stage2 = 2 (two dense 64x64 blocks), stage3 = 2
    ([128,64], two 64x32 blocks), stage4 (shortcut) = 1 ([128,128], four
    32x32 blocks) accumulated into the stage-3 PSUM.

Queue discipline: W2 streams as large chunk-pair DMAs on the sync HWDGE
ring only; x rides the gpsimd (SWDGE) ring at t=0; rs stores / RS
triggers / post-RS weight loads / output stores share the gpsimd queue in
dependency order; scalar+vector queues carry only PSUM->SBUF work
(activations, packs, merges), load-balanced ~73/27 for equal finish.
Output is stored fp16 and cast to f32 on the host.
"""

import contextlib

import numpy as np

import concourse.bacc as bacc
import concourse.mybir as mybir
import concourse.tile as tile
from concourse.bass_utils import run_bass_kernel_spmd

N_CORES = 8
B, LAT, FIN, FOUT, FH, H, W = 32, 256, 32, 32, 64, 128, 128
HW = H * W
K_IN, K_MID, K_OUT, K_SH = FH * FIN, FH * FH, FOUT * FH, FOUT * FIN
K_TOT = K_IN + K_MID + K_OUT + K_SH + FH + FH + FOUT + FOUT  # 9408
SHARD = K_TOT // N_CORES  # 1176 hypernet contraction rows per core
KP = SHARD + 1            # + one bias row (b2, on core 0 only)
KPAD = 1280               # h length padded to 10 chunks of 128
NCH = KPAD // 128         # 10
BPC = B // N_CORES        # 4 samples per core
NW = HW // 512            # 32 conv pixel windows
NSW = NW // 2             # 16 super-windows (2 windows / PSUM tile)
F32 = mybir.dt.float32
FP16 = mybir.dt.float16

# original ks offsets
OFF_IN, OFF_MID = 0, K_IN
OFF_OUT, OFF_SHC = K_IN + K_MID, K_IN + K_MID + K_OUT
OFF_B = OFF_SHC + K_SH  # 9216: b_in 64 | b_mid 64 | b_out 32 | b_short 32

# permuted ks layout: | biases 192 | k_inT 2048 | k_midT 4096 | k_shortT 1024
# | k_outT 2048 | — segments ordered by phase-B consumption.
NB_IN, NB_MID, NB_OUT, NB_SH = 0, 64, 128, 160
NK_IN = 192
SEGS = [(0, 2240), (2240, 4096), (6336, 3072)]
# W2 row-chunk DMA groups: (row0, nrows) pairs; last two are the ragged tail
GROUPS = [(0, 256), (256, 256), (512, 256), (768, 256), (1024, 128),
          (1152, 25)]

_CACHE: dict = {}


def _strips(wseg):
    return [(t, min(512, wseg - t)) for t in range(0, wseg, 512)]


def _build():
    nc = bacc.Bacc("TRN2", target_bir_lowering=False, num_devices=N_CORES)
    AF = mybir.ActivationFunctionType
    ALU = mybir.AluOpType

    xs = nc.dram_tensor("xs", [BPC * FIN, HW], FP16, kind="ExternalInput")
    latT = nc.dram_tensor("latT", [LAT, B], FP16, kind="ExternalInput")
    w1s = nc.dram_tensor("w1s", [LAT, KPAD], FP16, kind="ExternalInput")
    b1s = nc.dram_tensor("b1s", [128, NCH], F32, kind="ExternalInput")
    w2s = nc.dram_tensor("w2s", [KP, K_TOT], FP16, kind="ExternalInput")
    out = nc.dram_tensor("out", [BPC * FOUT, HW], FP16, kind="ExternalOutput")

    rs_in = [nc.dram_tensor(f"rs_in{s}", [B, w], FP16)
             for s, (_, w) in enumerate(SEGS)]
    ks_own = [nc.dram_tensor(f"ks_own{s}", [BPC, w], FP16)
              for s, (_, w) in enumerate(SEGS)]

    with tile.TileContext(nc) as tc, contextlib.ExitStack() as ctx:
        sing = ctx.enter_context(tc.tile_pool(name="sing", bufs=1))
        w2pool = ctx.enter_context(tc.tile_pool(name="w2p", bufs=2))
        kspool = ctx.enter_context(tc.tile_pool(name="ksp", bufs=2))
        outp = ctx.enter_context(tc.tile_pool(name="outp", bufs=2))
        # PSUM budget 8 banks of [128, 512] f32: psK 2 + pc (3 x 2-bank) 6.
        psK = ctx.enter_context(tc.tile_pool(name="psK", bufs=2, space="PSUM"))
        pc = ctx.enter_context(tc.tile_pool(name="pc", bufs=3, space="PSUM"))

        latT_sb = sing.tile([128, 2 * B], FP16)
        w1_sb = sing.tile([128, 2 * KPAD], FP16)
        b1_sb = sing.tile([128, NCH], F32)
        hT_sb = sing.tile([128, NCH * B], FP16)
        x4 = sing.tile([128, HW], FP16)
        y1all = sing.tile([128, 2 * HW], FP16)
        y2all = sing.tile([128, 2 * HW], FP16)

        # block-diagonal conv weight tiles (built from ks_own after each RS)
        kA1 = sing.tile([128, 128], FP16)   # s1: samples 0,1 (32x64 blocks)
        kB1 = sing.tile([128, 128], FP16)   # s1: samples 2,3
        kA2 = sing.tile([128, 128], FP16)   # s2: samples 0,1 (dense 64x64)
        kB2 = sing.tile([128, 128], FP16)   # s2: samples 2,3
        kA3 = sing.tile([128, 64], FP16)    # s3: samples 0,1 (64x32 blocks)
        kB3 = sing.tile([128, 64], FP16)    # s3: samples 2,3
        kC4 = sing.tile([128, 128], FP16)   # s4: 4 diag 32x32 blocks
        bin01 = sing.tile([128, 1], F32)
        bin23 = sing.tile([128, 1], F32)
        bmid01 = sing.tile([128, 1], F32)
        bmid23 = sing.tile([128, 1], F32)
        bout4 = sing.tile([128, 1], F32)
        bsh4 = sing.tile([128, 1], F32)
        bso = sing.tile([128, 1], F32)

        # zero the off-diagonal parts of the sparse lhsT tiles (vector, t~0)
        for t in (kA1, kB1, kA2, kB2, kA3, kB3, kC4):
            nc.vector.memset(t, 0)

        # x rides the gpsimd SWDGE ring, concurrent with W2 on sync HWDGE
        for xh in range(2):
            nc.gpsimd.dma_start(out=x4[:, 8192 * xh:8192 * (xh + 1)],
                                in_=xs[:, 8192 * xh:8192 * (xh + 1)])

        for l in range(2):
            nc.sync.dma_start(out=latT_sb[:, l * B:(l + 1) * B],
                              in_=latT[128 * l:128 * (l + 1), :])
            nc.sync.dma_start(out=w1_sb[:, l * KPAD:(l + 1) * KPAD],
                              in_=w1s[128 * l:128 * (l + 1), :])
        nc.sync.dma_start(out=b1_sb[:, :], in_=b1s[:, :])

        # ---- hypernet stage 1: hT[kcol, b] = sum_l W1[l, kcol]*lat[b, l] ----
        for c in range(NCH):
            ph = pc.tile([128, 1024], F32, tag="pc", name=f"ph{c}")
            for l in range(2):
                nc.tensor.matmul(
                    ph[:, 0:B],
                    lhsT=w1_sb[:, l * KPAD + 128 * c: l * KPAD + 128 * c + 128],
                    rhs=latT_sb[:, l * B:(l + 1) * B],
                    start=(l == 0), stop=(l == 1))
            nc.scalar.activation(out=hT_sb[:, c * B:(c + 1) * B], in_=ph[:, 0:B],
                                 func=AF.Identity, bias=b1_sb[:, c:c + 1],
                                 scale=1.0)

        # ---- W2 streaming DMAs (sync ring only) + strip matmuls ----
        seg_banks = []
        for s, (o0, wseg) in enumerate(SEGS):
            strips = _strips(wseg)
            nbank = (len(strips) + 3) // 4
            banks = [psK.tile([128, 512], F32, tag="pk", name=f"pk_{s}_{bi}")
                     for bi in range(nbank)]
            seg_banks.append(banks)
            for g, (r0, nr) in enumerate(GROUPS):
                ngc = (nr + 127) // 128  # chunks in this group (2 or 1)
                w2t = w2pool.tile([128, ngc * wseg], FP16, tag="w2t",
                                  name=f"w2t_{s}_{g}")
                if ngc == 2:
                    nc.sync.dma_start(
                        out=w2t.rearrange("p (c w) -> p c w", c=2),
                        in_=w2s[r0:r0 + nr, o0:o0 + wseg]
                        .rearrange("(c p) w -> p c w", p=128))
                else:
                    nc.sync.dma_start(out=w2t[0:nr, :],
                                      in_=w2s[r0:r0 + nr, o0:o0 + wseg])
                for lc in range(ngc):
                    c = r0 // 128 + lc
                    kc = min(128, nr - 128 * lc)
                    for j, (t0, tw) in enumerate(strips):
                        bank, sib = banks[j // 4], j % 4
                        nc.tensor.matmul(
                            bank[32 * sib:32 * sib + 32, 0:tw],
                            lhsT=hT_sb[0:kc, c * B:(c + 1) * B],
                            rhs=w2t[0:kc, lc * wseg + t0:lc * wseg + t0 + tw],
                            start=(c == 0), stop=(c == NCH - 1),
                            tile_position=(0, 32 * sib))

            # pack psum -> fp16 and store to rs_in (stores on gpsimd ring)
            peng = nc.scalar if s != 1 else nc.vector
            for bi, bank in enumerate(banks):
                bw = min(512, wseg - 2048 * bi)
                ks16 = kspool.tile([128, 512], FP16, tag="ks16",
                                   name=f"ks16_{s}_{bi}")
                if peng is nc.scalar:
                    nc.scalar.activation(out=ks16[:, 0:bw], in_=bank[:, 0:bw],
                                         func=AF.Copy, scale=1.0)
                else:
                    nc.vector.tensor_copy(ks16[:, 0:bw], bank[:, 0:bw])
                for j, (t0, tw) in enumerate(strips):
                    if j // 4 != bi:
                        continue
                    nc.gpsimd.dma_start(
                        out=rs_in[s][:, t0:t0 + tw],
                        in_=ks16[32 * (j % 4):32 * (j % 4) + 32, 0:tw])
            nc.gpsimd.collective_compute(
                "ReduceScatter", ALU.add,
                replica_groups=[list(range(N_CORES))],
                ins=[rs_in[s][:, :].opt()], outs=[ks_own[s][:, :].opt()])

            # post-RS weight loads for this segment (gpsimd, after trigger)
            if s == 0:
                for q in range(2):
                    nc.gpsimd.dma_start(
                        out=kA1[32 * q:32 * q + 32, 64 * q:64 * q + 64],
                        in_=ks_own[0][q:q + 1, NK_IN:NK_IN + K_IN]
                        .rearrange("q (i o) -> (q i) o", i=FIN))
                for q in range(2):
                    nc.gpsimd.dma_start(
                        out=kB1[64 + 32 * q:96 + 32 * q, 64 * q:64 * q + 64],
                        in_=ks_own[0][q + 2:q + 3, NK_IN:NK_IN + K_IN]
                        .rearrange("q (i o) -> (q i) o", i=FIN))
                nc.gpsimd.dma_start(out=bin01,
                                    in_=ks_own[0][0:2, NB_IN:NB_IN + FH])
                nc.gpsimd.dma_start(out=bin23,
                                    in_=ks_own[0][2:4, NB_IN:NB_IN + FH])
                nc.gpsimd.dma_start(out=bmid01,
                                    in_=ks_own[0][0:2, NB_MID:NB_MID + FH])
                nc.gpsimd.dma_start(out=bmid23,
                                    in_=ks_own[0][2:4, NB_MID:NB_MID + FH])
                nc.gpsimd.dma_start(out=bout4,
                                    in_=ks_own[0][0:4, NB_OUT:NB_OUT + FOUT])
                nc.gpsimd.dma_start(out=bsh4,
                                    in_=ks_own[0][0:4, NB_SH:NB_SH + FOUT])
            elif s == 1:
                for q in range(2):
                    nc.gpsimd.dma_start(
                        out=kA2[64 * q:64 * q + 64, 64 * q:64 * q + 64],
                        in_=ks_own[1][q:q + 1, 0:K_MID]
                        .rearrange("q (i o) -> (q i) o", i=FH))
                for q in range(2):
                    nc.gpsimd.dma_start(
                        out=kB2[64 * q:64 * q + 64, 64 * q:64 * q + 64],
                        in_=ks_own[1][q + 2:q + 3, 0:K_MID]
                        .rearrange("q (i o) -> (q i) o", i=FH))
            else:
                for q in range(2):
                    nc.gpsimd.dma_start(
                        out=kA3[64 * q:64 * q + 64, 32 * q:32 * q + 32],
                        in_=ks_own[2][q:q + 1, K_SH:K_SH + K_OUT]
                        .rearrange("q (i o) -> (q i) o", i=FH))
                for q in range(2):
                    nc.gpsimd.dma_start(
                        out=kB3[64 * q:64 * q + 64, 32 * q:32 * q + 32],
                        in_=ks_own[2][q + 2:q + 3, K_SH:K_SH + K_OUT]
                        .rearrange("q (i o) -> (q i) o", i=FH))
                for q in range(4):
                    nc.gpsimd.dma_start(
                        out=kC4[32 * q:32 * q + 32, 32 * q:32 * q + 32],
                        in_=ks_own[2][q:q + 1, 0:K_SH]
                        .rearrange("q (i o) -> (q i) o", i=FIN))

        nc.vector.tensor_add(bso, bout4, bsh4)

        # ---- conv phase emit helpers ------------------------------------
        # y1all/y2all column layout: window w -> [1024w, 1024w+512) holds the
        # A half (samples 0,1 as 64+64 rows), [+512, +1024) the B half (2,3).
        act_ctr = [0]

        def lrelu_pair(psum_t, yall, sw, half, bias):
            # one [128,1024] psum tile = halves of windows 2sw and 2sw+1
            dst = yall.rearrange("p (w h c) -> p w h c", h=2, c=512)[
                :, 2 * sw:2 * sw + 2, half, :]
            src = psum_t.rearrange("p (w c) -> p w c", c=512)
            # ~73% of tiles to scalar (1-op biased lrelu), rest to vector
            act_ctr[0] += 1
            if act_ctr[0] % 4 != 0:
                nc.scalar.activation(out=dst, in_=src, func=AF.Lrelu,
                                     bias=bias[:, 0:1], scale=1.0, alpha=0.01)
            else:
                nc.vector.tensor_scalar(out=dst, in0=src,
                                        scalar1=bias[:, 0:1], scalar2=None,
                                        op0=ALU.add)
                nc.vector.scalar_tensor_tensor(
                    out=dst, in0=dst, scalar=0.01, in1=dst,
                    op0=ALU.mult, op1=ALU.max)

        def emit_s1(sw):
            pA = pc.tile([128, 1024], F32, tag="pc", name=f"p1a_{sw}")
            pB = pc.tile([128, 1024], F32, tag="pc", name=f"p1b_{sw}")
            for k in range(2):
                c0 = 512 * (2 * sw + k)
                nc.tensor.matmul(pA[:, 512 * k:512 * k + 512], lhsT=kA1,
                                 rhs=x4[:, c0:c0 + 512], start=True, stop=True)
                nc.tensor.matmul(pB[:, 512 * k:512 * k + 512], lhsT=kB1,
                                 rhs=x4[:, c0:c0 + 512], start=True, stop=True)
            lrelu_pair(pA, y1all, sw, 0, bin01)
            lrelu_pair(pB, y1all, sw, 1, bin23)

        def emit_s2(sw):
            pA = pc.tile([128, 1024], F32, tag="pc", name=f"p2a_{sw}")
            pB = pc.tile([128, 1024], F32, tag="pc", name=f"p2b_{sw}")
            for k in range(2):
                yc0 = 1024 * (2 * sw + k)
                nc.tensor.matmul(pA[:, 512 * k:512 * k + 512], lhsT=kA2,
                                 rhs=y1all[:, yc0:yc0 + 512],
                                 start=True, stop=True)
                nc.tensor.matmul(pB[:, 512 * k:512 * k + 512], lhsT=kB2,
                                 rhs=y1all[:, yc0 + 512:yc0 + 1024],
                                 start=True, stop=True)
            lrelu_pair(pA, y2all, sw, 0, bmid01)
            lrelu_pair(pB, y2all, sw, 1, bmid23)

        def emit_s34(sw):
            p3 = pc.tile([128, 1024], F32, tag="pc", name=f"p3_{sw}")
            for k in range(2):
                w = 2 * sw + k
                yc0, c0 = 1024 * w, 512 * w
                dst = p3[:, 512 * k:512 * k + 512]
                nc.tensor.matmul(dst[0:64, :], lhsT=kA3,
                                 rhs=y2all[:, yc0:yc0 + 512],
                                 start=True, stop=False, tile_position=(0, 0))
                nc.tensor.matmul(dst[64:128, :], lhsT=kB3,
                                 rhs=y2all[:, yc0 + 512:yc0 + 1024],
                                 start=True, stop=False, tile_position=(0, 64))
                nc.tensor.matmul(dst, lhsT=kC4, rhs=x4[:, c0:c0 + 512],
                                 start=False, stop=True)
            o_sb = outp.tile([128, 1024], FP16, tag="o", name=f"o_{sw}")
            nc.vector.tensor_scalar(out=o_sb, in0=p3, scalar1=bso[:, 0:1],
                                    scalar2=None, op0=ALU.add)
            nc.gpsimd.dma_start(out=out[:, 1024 * sw:1024 * sw + 1024],
                                in_=o_sb)

        # s1 for all super-windows (ready ~RS0-done), then s2, then s3/s4
        for sw in range(NSW):
            emit_s1(sw)
        for sw in range(NSW):
            emit_s2(sw)
        for sw in range(NSW):
            emit_s34(sw)

    nc.compile()
    return nc


def _seg_perm(rows, cols):
    # new position (c, r) holds old flat index r*cols + c
    return np.arange(rows * cols).reshape(rows, cols).T.ravel()


def _perm():
    # permutation of ks columns: conv kernels arrive transposed (lhsT layout)
    # and segments reordered to the phase-B consumption order
    return np.concatenate([
        np.arange(OFF_B, K_TOT),          # biases first
        OFF_IN + _seg_perm(FH, FIN),      # k_inT
        OFF_MID + _seg_perm(FH, FH),      # k_midT
        OFF_SHC + _seg_perm(FOUT, FIN),   # k_shortT
        OFF_OUT + _seg_perm(FOUT, FH),    # k_outT
    ])


def _prep_in_maps(x, lat, W1, b1, W2, b2):
    x = np.ascontiguousarray(x, np.float32)
    lat = np.ascontiguousarray(lat, np.float32)
    W1 = np.ascontiguousarray(W1, np.float32)
    b1 = np.asarray(b1, np.float32)
    W2 = np.asarray(W2, np.float32)
    b2 = np.asarray(b2, np.float32)

    perm = _perm()
    W2p = np.ascontiguousarray(W2[:, perm]).astype(np.float16)
    b2p = b2[perm].astype(np.float16)
    latT = np.ascontiguousarray(lat.T).astype(np.float16)
    xr = x.reshape(B, FIN, HW)

    in_maps = []
    for i in range(N_CORES):
        sh = slice(i * SHARD, (i + 1) * SHARD)
        w1p = np.zeros((LAT, KPAD), np.float16)
        w1p[:, :SHARD] = W1[:, sh]
        b1p = np.zeros((KPAD,), np.float32)
        b1p[:SHARD] = b1[sh]
        b1p[SHARD] = 1.0  # the "ones" h-slot that carries b2
        w2a = np.zeros((KP, K_TOT), np.float16)
        w2a[:SHARD] = W2p[sh]
        if i == 0:
            w2a[SHARD] = b2p
        xsi = np.ascontiguousarray(
            xr[4 * i:4 * i + 4].reshape(BPC * FIN, HW)).astype(np.float16)
        in_maps.append({
            "xs": xsi,
            "latT": latT,
            "w1s": w1p,
            "b1s": np.ascontiguousarray(b1p.reshape(NCH, 128).T),
            "w2s": w2a,
        })
    return in_maps


def _run(in_maps, **kwargs):
    if "nc" not in _CACHE:
        _CACHE["nc"] = _build()
    return run_bass_kernel_spmd(_CACHE["nc"], in_maps,
                                core_ids=list(range(N_CORES)), **kwargs)


def _assemble(results):
    parts = [r["out"].astype(np.float32).reshape(BPC, FOUT, H, W)
             for r in results]
    return np.ascontiguousarray(np.concatenate(parts, axis=0))


def kernel(x, lat, W1, b1, W2, b2):
    in_maps = _prep_in_maps(x, lat, W1, b1, W2, b2)
    res = _run(in_maps)
    return _assemble(res.results)
